# revision 12
# baseline (speedup 1.0000x reference)
"""Multi-head attention with "restricted softmax" on 8 TRN2 NeuronCores.

Reference computation (per head):
    score = Q @ K.T / sqrt(D)                       # [S, S]
    attn  = exp(score) / (1 + sum_k exp(score))     # restricted softmax
            (mathematically identical to the max-clamped reference form)
    out   = attn @ V                                # [S, D]

Full problem: B=2, H=16, S=2048, D=64  ->  32 heads, 4 heads per core.

Per-core kernel strategy (no communication needed):
  - Scores computed TRANSPOSED (S^T[k, q]) so softmax's k-reduction sits on
    the PSUM partition axis where the PE performs it for free: PV uses
    lhsT=[V | 1] so the extra output row is sum_k exp = the denominator.
  - All matmul operands are fp16 with fp32 PSUM accumulation; the scores
    contraction (d=64) is ZERO-PADDED to K=128 (half-height weights block
    LDWEIGHTS pipelining: 427 vs 216 ns per 512-col matmul).
  - The ScalarEngine exp is the bottleneck (1 elem/cycle/lane, ~142us for
    the 16.8M exps per core).  4 of every 16 k-tiles are therefore computed
    on the otherwise-idle VectorEngine with a Schraudolph-style fp16
    bit-pattern exp: i16 = s*(log2e*1024/8) + (15*1024 - 59.3), bitcast to
    fp16 == exp(score/8) with a mean-centered +-2% mantissa-interpolation
    ripple (net output rel-err ~0.8%, inside the 2e-2 gate).
  - Q/K transposes ride the DMA X-bar via a DRAM bounce (fp16, d padded to
    128 so the zero rows come for free); head 0 is chunked so the first
    scores tile lands as early as possible.
  - Scores PSUM pool is 3-deep (6 banks) for loose exp pipelining.
  - The epilogue transposes oT back to [q, d] with a single 3D-dst DMA
    X-bar transpose per pass (no TensorEngine involvement), then
    normalizes on the VectorEngine.
"""

import math
import os

import numpy as np

import concourse.bass as bass  # noqa: F401  (bass must import before tile)
import concourse.mybir as mybir
import concourse.tile as tile
from concourse import bacc
from concourse.bass_utils import run_bass_kernel_spmd
from concourse.masks import make_identity

B, H, S, D = 2, 16, 2048, 64
N_CORES = 8
HPC = (B * H) // N_CORES  # heads per core = 4

F32 = mybir.dt.float32
F16 = mybir.dt.float16
I16 = mybir.dt.int16
EXP = mybir.ActivationFunctionType.Exp

SCALE = 1.0 / 8.0  # 1/sqrt(D)
NQ = S // 128      # 16 tiles of 128 along both q and k
QH = 1024          # q-half width processed per pass
NB = QH // 512     # 512-wide matmuls per scores tile

# k-tiles whose exp runs on the VectorEngine via the fp16 bit-pattern trick
OFF = (2, 6, 10, 14)
EXP_A = math.log2(math.e) * 1024.0 / 8.0          # 184.665 (includes 1/sqrt(D))
EXP_B = 15.0 * 1024.0 - 59.29                     # mean-centering constant


class _HeadInputs:
    """Per-head staged inputs: fp16 Q^T/K^T [128, S] (rows 0..63 data, rows
    64..127 exact zeros so the scores matmul contracts over K=128) and [V|1].

    Transposes run on the DMA X-bar through a DRAM bounce: fp32 load ->
    fp16 cast (d padded 64->128 with zeros) -> DRAM store -> one transposed
    load per chunk.  Zero PE cost; the d-padding produces the zero rows of
    qT/kT for free."""

    def __init__(self, ctx, h):
        self.ctx = ctx
        self.h = h
        self.tp_count = 0

    def emit_transpose(self, kind, n):
        """Head-0 ramp transpose on the TensorEngine.  PSUM is fully booked
        (3 score slots + oT), so the [D, 128] transpose outputs borrow space
        from the idle scores pool: an f16 bitcast view of a slice of an
        [128, QH] f32 "s"-tagged tile.  The pool's WAR tracking then orders
        the first real scores matmuls after these reads automatically."""
        nc, pools = self.ctx["nc"], self.ctx
        st16, tT = (self.q16, self.qT) if kind == "q" else (self.k16, self.kT)
        i = self.tp_count % 8
        if i == 0:
            self.tp_tile = pools["ps_s_pool"].tile(
                [128, QH], F32, tag="s", name="tp_host"
            )
        self.tp_count += 1
        tp = self.tp_tile[:D, i * 64:(i + 1) * 64].bitcast(F16)
        nc.tensor.transpose(tp, st16[:, n, :D], pools["ident16"][:])
        nc.vector.tensor_copy(tT[:D, n * 128:(n + 1) * 128], tp)

    def start_dma_split(self):
        """Head-0 ramp: PE is idle, so transpose on the TensorEngine
        (shorter critical chain than the DRAM bounce)."""
        nc, pools, h = self.ctx["nc"], self.ctx, self.h
        head_pool = pools["head_pool"]
        qkt_pool = pools["qkt_pool"]

        q_nat = head_pool.tile([128, NQ, D], F32, tag="q_nat", name=f"q_nat{h}")
        k_nat = head_pool.tile([128, NQ, D], F32, tag="k_nat", name=f"k_nat{h}")
        v_nat = head_pool.tile([128, NQ, D], F32, tag="v_nat", name=f"v_nat{h}")
        q16 = head_pool.tile([128, NQ, 128], F16, tag="q16", name=f"q16_{h}")
        k16 = head_pool.tile([128, NQ, 128], F16, tag="k16", name=f"k16_{h}")
        nc.gpsimd.memset(q16[:, :, D:], 0.0)
        nc.gpsimd.memset(k16[:, :, D:], 0.0)
        self.q16, self.k16 = q16, k16
        self.qT = qkt_pool.tile([128, S], F16, tag="qT", name=f"qT{h}")
        self.kT = qkt_pool.tile([128, S], F16, tag="kT", name=f"kT{h}")
        nc.gpsimd.memset(self.qT[D:, :], 0.0)
        nc.gpsimd.memset(self.kT[D:, :], 0.0)
        hn = NQ // 2
        for i in range(2):
            ns = slice(i * hn, (i + 1) * hn)
            for nat, st16, dram_src in (
                (q_nat, q16, pools["q_dram"]),
                (k_nat, k16, pools["k_dram"]),
            ):
                nc.sync.dma_start(
                    nat[:, ns, :],
                    dram_src[h].rearrange("(n p) d -> p n d", p=128)[:, ns, :],
                )
                nc.vector.tensor_copy(st16[:, ns, :D], nat[:, ns, :])
            if i == 0:
                nc.sync.dma_start(
                    v_nat[:],
                    pools["v_dram"][h].rearrange("(n p) d -> p n d", p=128),
                )
            if i == 0:
                # kT tile 0 + qT tiles 0-7 unblock the first scores matmul
                order = [("k", 0)] + [("q", n) for n in range(hn)] \
                    + [("k", n) for n in range(1, hn)]
            else:
                order = [(kind, n) for kind in ("q", "k")
                         for n in range(i * hn, (i + 1) * hn)]
            for kind, n in order:
                self.emit_transpose(kind, n)
            if i == 0:
                v1 = head_pool.tile([128, NQ, D + 1], F16, tag="v1", name=f"v1_{h}")
                nc.vector.tensor_copy(
                    v1[:, :, D:].rearrange("p n one -> p (n one)"),
                    pools["ones"][:],
                )
                nc.vector.tensor_copy(v1[:, :, :D], v_nat[:])
                self.v1 = v1

    def start_dma(self, groups=None):
        """Emit the staging pipeline.  All DMA triggers run on the in-order
        SP queue, so a trigger whose dependency is unmet BLOCKS every DMA
        behind it; the emission order below is arranged so each trigger's
        producer has (almost) always completed by the time SP reaches it.

        groups: list of lists of (kind, n0, n1) chunk descriptors; each
        group emits load -> cast -> store -> transpose for its chunks in
        dependency-pipelined order.  "v" entries load V and build [V|1].
        """
        nc, pools, h = self.ctx["nc"], self.ctx, self.h
        head_pool = pools["head_pool"]
        dram_pool = pools["dram_pool"]
        qkt_pool = pools["qkt_pool"]

        q_nat = head_pool.tile([128, NQ, D], F32, tag="q_nat", name=f"q_nat{h}")
        k_nat = head_pool.tile([128, NQ, D], F32, tag="k_nat", name=f"k_nat{h}")
        v_nat = head_pool.tile([128, NQ, D], F32, tag="v_nat", name=f"v_nat{h}")
        # fp16 staging with d padded 64->128; pad columns stay zero across
        # slot reuse (each head only rewrites cols 0..63), cleared on the
        # first two uses (one per pool slot).
        q16 = head_pool.tile([128, NQ, 128], F16, tag="q16", name=f"q16_{h}")
        k16 = head_pool.tile([128, NQ, 128], F16, tag="k16", name=f"k16_{h}")
        if h < 2:
            # gpsimd (idle engine) so the DVE cast chain isn't delayed
            nc.gpsimd.memset(q16[:, :, D:], 0.0)
            nc.gpsimd.memset(k16[:, :, D:], 0.0)
        qdr = dram_pool.tile([S, 128], F16, tag="qdr", name=f"qdr{h}")
        kdr = dram_pool.tile([S, 128], F16, tag="kdr", name=f"kdr{h}")
        self.qT = qkt_pool.tile([128, S], F16, tag="qT", name=f"qT{h}")
        self.kT = qkt_pool.tile([128, S], F16, tag="kT", name=f"kT{h}")

        if groups is None:
            groups = [[("q", 0, NQ), ("k", 0, NQ), ("v", 0, NQ)]]

        def tensors(kind):
            return {
                "q": (q_nat, q16, qdr, self.qT, pools["q_dram"]),
                "k": (k_nat, k16, kdr, self.kT, pools["k_dram"]),
            }[kind]

        for group in groups:
            # stage 1: loads (wait-free on SP) + casts (DVE)
            for kind, n0, n1 in group:
                if kind == "v":
                    nc.sync.dma_start(
                        v_nat[:],
                        pools["v_dram"][h].rearrange("(n p) d -> p n d", p=128),
                    )
                    v1 = head_pool.tile([128, NQ, D + 1], F16, tag="v1", name=f"v1_{h}")
                    nc.vector.tensor_copy(
                        v1[:, :, D:].rearrange("p n one -> p (n one)"),
                        pools["ones"][:],
                    )
                    nc.vector.tensor_copy(v1[:, :, :D], v_nat[:])
                    self.v1 = v1
                    continue
                nat, st16, dr, tT, dram_src = tensors(kind)
                ns = slice(n0, n1)
                nc.sync.dma_start(
                    nat[:, ns, :],
                    dram_src[h].rearrange("(n p) d -> p n d", p=128)[:, ns, :],
                )
                nc.vector.tensor_copy(st16[:, ns, :D], nat[:, ns, :])
            # stage 2: bounce stores (wait on casts, usually met)
            for kind, n0, n1 in group:
                if kind == "v":
                    continue
                nat, st16, dr, tT, dram_src = tensors(kind)
                ns = slice(n0, n1)
                nc.sync.dma_start(
                    dr[:].rearrange("(n p) c -> p n c", p=128)[:, ns, :],
                    st16[:, ns, :],
                )
            # stage 3: X-bar transposes (wait on stores, usually met)
            for kind, n0, n1 in group:
                if kind == "v":
                    continue
                nat, st16, dr, tT, dram_src = tensors(kind)
                rs = slice(n0 * 128, n1 * 128)
                nc.sync.dma_start_transpose(tT[:, rs], dr[rs, :])


def _attention(tc):
    nc = tc.nc
    q_dram = nc.dram_tensor("query", [HPC, S, D], F32, kind="ExternalInput").ap()
    k_dram = nc.dram_tensor("key", [HPC, S, D], F32, kind="ExternalInput").ap()
    v_dram = nc.dram_tensor("value", [HPC, S, D], F32, kind="ExternalInput").ap()
    o_dram = nc.dram_tensor("out", [HPC, S, D], F32, kind="ExternalOutput").ap()

    with (
        tc.tile_pool(name="const", bufs=1) as const_pool,
        tc.tile_pool(name="head_io", bufs=2) as head_pool,
        tc.tile_pool(name="qkt", bufs=2) as qkt_pool,
        tc.tile_pool(name="et", bufs=4) as et_pool,
        tc.tile_pool(name="eti", bufs=2) as eti_pool,
        tc.tile_pool(name="epi", bufs=2) as epi_pool,
        tc.tile_pool(name="dram", bufs=2, space="DRAM") as dram_pool,
        tc.tile_pool(name="ps_s", bufs=3, space="PSUM") as ps_s_pool,
        tc.tile_pool(name="ps_o", bufs=1, space="PSUM") as ps_o_pool,
    ):
        ident16 = const_pool.tile([128, 128], F16)
        make_identity(nc, ident16[:])
        ones = const_pool.tile([128, NQ], F16)
        nc.vector.memset(ones[:], 1.0)

        ctx = {
            "nc": nc, "q_dram": q_dram, "k_dram": k_dram, "v_dram": v_dram,
            "head_pool": head_pool, "qkt_pool": qkt_pool,
            "ps_s_pool": ps_s_pool, "dram_pool": dram_pool,
            "ident16": ident16, "ones": ones,
        }

        heads = [_HeadInputs(ctx, h) for h in range(HPC)]
        heads[0].start_dma_split()

        def emit_scores(hd, qh, k, s_ps):
            for b in range(NB):
                q0 = qh * QH + b * 512
                nc.tensor.matmul(
                    s_ps[:, b * 512:(b + 1) * 512],
                    hd.kT[:, k * 128:(k + 1) * 128],
                    hd.qT[:, q0:q0 + 512],
                    start=True, stop=True,
                )

        def emit_pv(hd, oT, k, et_ap):
            for b in range(NB):
                nc.tensor.matmul(
                    oT[:, b * 512:(b + 1) * 512],
                    hd.v1[:, k, :],
                    et_ap[:, b * 512:(b + 1) * 512],
                    start=(k == 0), stop=(k == NQ - 1),
                )

        def epi_copy(st):
            """Stage A: evict oT PSUM -> fp16 SBUF (DVE)."""
            st["oT16"] = epi_pool.tile([80, QH], F16, tag="oT16", name="oT16")
            nc.vector.tensor_copy(st["oT16"][:D + 1, :], st["oT"][:])

        def epi_transpose(st):
            """Stage B: one X-bar transpose [80, QH] -> [128, 8, 80]."""
            st["trT"] = epi_pool.tile([128, QH // 128, 80], F16, tag="trT", name="trT")
            nc.sync.dma_start_transpose(st["trT"][:], st["oT16"][:])

        def epi_normalize(st):
            """Stage C: per-q reciprocal normalize (DVE) + output DMA."""
            h, qh, trT = st["h"], st["qh"], st["trT"]
            den = epi_pool.tile([128, QH // 128], F32, tag="den", name="den")
            nc.vector.tensor_scalar_add(den[:], trT[:, :, D], 1.0)
            rec = epi_pool.tile([128, QH // 128], F32, tag="rec", name="rec")
            nc.vector.reciprocal(rec[:], den[:])
            o_sb = epi_pool.tile([128, QH // 128, D], F32, tag="o_sb", name="o_sb")
            for j in range(QH // 128):
                nc.vector.tensor_scalar_mul(o_sb[:, j, :], trT[:, j, :D], rec[:, j:j + 1])
            nc.sync.dma_start(
                o_dram[h].rearrange("(n p) d -> p n d", p=128)[:, qh * 8:(qh + 1) * 8, :],
                o_sb[:],
            )

        pending_epi = []
        passes = [(h, qh) for h in range(HPC) for qh in range(S // QH)]
        s_carry = None
        for idx, (h, qh) in enumerate(passes):
            hd = heads[h]
            # prefetch the next head's staging a full pass ahead
            if qh == 0 and h + 1 < HPC:
                heads[h + 1].start_dma()

            oT = ps_o_pool.tile([D + 1, QH], F32, tag="oT", name="oT")
            s_tiles = {}
            if s_carry is not None:
                s_tiles[0] = s_carry
                s_carry = None
            else:
                s_tiles[0] = ps_s_pool.tile([128, QH], F32, tag="s", name="s0")
                emit_scores(hd, qh, 0, s_tiles[0])
            for k in range(NQ):
                if k in OFF:
                    eti = eti_pool.tile([128, QH], I16, tag="eti", name=f"eti{k}")
                    nc.vector.tensor_scalar(
                        eti[:], s_tiles[k][:], EXP_A, EXP_B,
                        mybir.AluOpType.mult, mybir.AluOpType.add,
                    )
                    et_ap = eti[:].bitcast(F16)
                else:
                    et = et_pool.tile([128, QH], F16, tag="et", name=f"et{k}")
                    nc.scalar.activation(et[:], s_tiles[k][:], EXP, scale=SCALE)
                    et_ap = et[:]
                if k + 1 < NQ:
                    s_tiles[k + 1] = ps_s_pool.tile([128, QH], F32, tag="s", name=f"s{k + 1}")
                    emit_scores(hd, qh, k + 1, s_tiles[k + 1])
                elif idx + 1 < len(passes):
                    # hoist the next pass's first scores into this pass's
                    # tail so the ScalarEngine never idles at the boundary
                    nh, nqh = passes[idx + 1]
                    s_carry = ps_s_pool.tile([128, QH], F32, tag="s", name="sc")
                    emit_scores(heads[nh], nqh, 0, s_carry)
                # drain the previous pass's epilogue in stages (placed just
                # after OFF tiles so the DVE work never delays an eti) so
                # each DMA trigger's dependency is met when the in-order SP
                # queue reaches it
                if pending_epi:
                    if k == 3:
                        epi_copy(pending_epi[0])
                    elif k == 5:
                        epi_transpose(pending_epi[0])
                    elif k == 8:
                        epi_normalize(pending_epi.pop(0))
                emit_pv(hd, oT, k, et_ap)
                del s_tiles[k]
            pending_epi.append({"h": h, "qh": qh, "oT": oT})
        for st in pending_epi:
            epi_copy(st)
            epi_transpose(st)
            epi_normalize(st)


_NC_CACHE = None
_TRACE_READY = False


def _enable_tracing():
    """Register the NTFF profile hook that this image's antenv lacks, and
    keep profiling artifacts local instead of uploading to a bucket."""
    global _TRACE_READY
    if _TRACE_READY:
        return
    import sys
    import types

    import antenv
    import concourse.bass_utils as bu
    from trn_agent_boot.trn_boot import _ntff_profile_via_ctypes

    if "antenv.axon_hooks" not in sys.modules:
        mod = types.ModuleType("antenv.axon_hooks")
        mod._hook = None

        def set_axon_ntff_profile_hook(h):
            mod._hook = h

        def get_axon_ntff_profile_hook():
            return mod._hook

        mod.set_axon_ntff_profile_hook = set_axon_ntff_profile_hook
        mod.get_axon_ntff_profile_hook = get_axon_ntff_profile_hook
        sys.modules["antenv.axon_hooks"] = mod
        antenv.axon_hooks = mod

    hooks = sys.modules["antenv.axon_hooks"]
    if hooks.get_axon_ntff_profile_hook() is None:
        hooks.set_axon_ntff_profile_hook(
            _ntff_profile_via_ctypes("/opt/axon/libaxon_pjrt.so")
        )
    bu.upload_artifacts = lambda tmpdir: tmpdir
    _TRACE_READY = True


def _build():
    global _NC_CACHE
    if _NC_CACHE is None:
        nc = bacc.Bacc("TRN2", target_bir_lowering=False, debug=False)
        with tile.TileContext(nc) as tc:
            _attention(tc)
        nc.compile()
        _NC_CACHE = nc
    return _NC_CACHE


def _run(query, key, value, trace=False, tmpdir=None):
    if trace:
        _enable_tracing()
    q = np.ascontiguousarray(np.asarray(query, dtype=np.float32).reshape(B * H, S, D))
    k = np.ascontiguousarray(np.asarray(key, dtype=np.float32).reshape(B * H, S, D))
    v = np.ascontiguousarray(np.asarray(value, dtype=np.float32).reshape(B * H, S, D))
    in_maps = [
        {
            "query": q[c * HPC:(c + 1) * HPC],
            "key": k[c * HPC:(c + 1) * HPC],
            "value": v[c * HPC:(c + 1) * HPC],
        }
        for c in range(N_CORES)
    ]
    nc = _build()
    res = run_bass_kernel_spmd(
        nc, in_maps, core_ids=list(range(N_CORES)), trace=trace, tmpdir=tmpdir
    )
    out = np.stack([res.results[c]["out"] for c in range(N_CORES)])  # [8, HPC, S, D]
    return out.reshape(B, H, S, D), res


def kernel(query, key, value):
    out, _ = _run(query, key, value, trace=bool(int(os.environ.get("BASS_TRACE", "0"))))
    return out


# revision 13
# speedup vs baseline: 1.1347x; 1.1347x over previous
"""Multi-head attention with "restricted softmax" on 8 TRN2 NeuronCores.

Reference computation (per head):
    score = Q @ K.T / sqrt(D)                       # [S, S]
    attn  = exp(score) / (1 + sum_k exp(score))     # restricted softmax
            (mathematically identical to the max-clamped reference form)
    out   = attn @ V                                # [S, D]

Full problem: B=2, H=16, S=2048, D=64  ->  32 heads, 4 heads per core.

Per-core kernel strategy (no communication needed):
  - Scores computed TRANSPOSED (S^T[k, q]) so softmax's k-reduction sits on
    the PSUM partition axis where the PE performs it for free: PV uses
    lhsT=[V | 1] so the extra output row is sum_k exp = the denominator.
  - All matmul operands are fp16 with fp32 PSUM accumulation; the scores
    contraction (d=64) is ZERO-PADDED to K=128 (half-height weights block
    LDWEIGHTS pipelining: 427 vs 216 ns per 512-col matmul).
  - The ScalarEngine exp is the bottleneck (1 elem/cycle/lane, ~142us for
    the 16.8M exps per core).  4 of every 16 k-tiles are therefore computed
    on the otherwise-idle VectorEngine with a Schraudolph-style fp16
    bit-pattern exp: i16 = s*(log2e*1024/8) + (15*1024 - 59.3), bitcast to
    fp16 == exp(score/8) with a mean-centered +-2% mantissa-interpolation
    ripple (net output rel-err ~0.8%, inside the 2e-2 gate).
  - Q/K transposes ride the DMA X-bar via a DRAM bounce (fp16, d padded to
    128 so the zero rows come for free); head 0 is chunked so the first
    scores tile lands as early as possible.
  - Scores PSUM pool is 3-deep (6 banks) for loose exp pipelining.
  - The epilogue transposes oT back to [q, d] with a single 3D-dst DMA
    X-bar transpose per pass (no TensorEngine involvement), then
    normalizes on the VectorEngine.
"""

import math
import os

import numpy as np

import concourse.bass as bass  # noqa: F401  (bass must import before tile)
import concourse.mybir as mybir
import concourse.tile as tile
from concourse import bacc
from concourse.bass_utils import run_bass_kernel_spmd
from concourse.masks import make_identity

B, H, S, D = 2, 16, 2048, 64
N_CORES = 8
HPC = (B * H) // N_CORES  # heads per core = 4

F32 = mybir.dt.float32
F16 = mybir.dt.float16
I16 = mybir.dt.int16
EXP = mybir.ActivationFunctionType.Exp

SCALE = 1.0 / 8.0  # 1/sqrt(D)
NQ = S // 128      # 16 tiles of 128 along both q and k
QH = 1024          # q-half width processed per pass
NB = QH // 512     # 512-wide matmuls per scores tile

# k-tiles whose exp runs on the VectorEngine via the fp16 bit-pattern trick
OFF = (2, 6, 10, 14)
EXP_A = math.log2(math.e) * 1024.0 / 8.0          # 184.665 (includes 1/sqrt(D))
EXP_B = 15.0 * 1024.0 - 59.29                     # mean-centering constant


class _HeadInputs:
    """Per-head staged inputs: fp16 Q^T/K^T [128, S] (rows 0..63 data, rows
    64..127 exact zeros so the scores matmul contracts over K=128) and [V|1].

    Transposes run on the DMA X-bar through a DRAM bounce: fp32 load ->
    fp16 cast (d padded 64->128 with zeros) -> DRAM store -> one transposed
    load per chunk.  Zero PE cost; the d-padding produces the zero rows of
    qT/kT for free."""

    def __init__(self, ctx, h):
        self.ctx = ctx
        self.h = h
        self.tp_count = 0

    def emit_transpose(self, kind, n):
        """Head-0 ramp transpose on the TensorEngine.  PSUM is fully booked
        (3 score slots + oT), so the [D, 128] transpose outputs borrow space
        from the idle scores pool: an f16 bitcast view of a slice of an
        [128, QH] f32 "s"-tagged tile.  The pool's WAR tracking then orders
        the first real scores matmuls after these reads automatically."""
        nc, pools = self.ctx["nc"], self.ctx
        st16, tT = (self.q16, self.qT) if kind == "q" else (self.k16, self.kT)
        i = self.tp_count % 8
        if i == 0:
            self.tp_tile = pools["ps_f_pool"].tile(
                [128, QH], F32, tag="f", name="tp_host"
            )
        self.tp_count += 1
        tp = self.tp_tile[:D, i * 64:(i + 1) * 64].bitcast(F16)
        nc.tensor.transpose(tp, st16[:, n, :D], pools["ident16"][:])
        nc.vector.tensor_copy(tT[:D, n * 128:(n + 1) * 128], tp)

    def start_dma_split(self):
        """Head-0 ramp: PE is idle, so transpose on the TensorEngine
        (shorter critical chain than the DRAM bounce)."""
        nc, pools, h = self.ctx["nc"], self.ctx, self.h
        head_pool = pools["head_pool"]
        qkt_pool = pools["qkt_pool"]

        q_nat = head_pool.tile([128, NQ, D], F32, tag="q_nat", name=f"q_nat{h}")
        k_nat = head_pool.tile([128, NQ, D], F32, tag="k_nat", name=f"k_nat{h}")
        v_nat = head_pool.tile([128, NQ, D], F32, tag="v_nat", name=f"v_nat{h}")
        q16 = head_pool.tile([128, NQ, 128], F16, tag="q16", name=f"q16_{h}")
        k16 = head_pool.tile([128, NQ, 128], F16, tag="k16", name=f"k16_{h}")
        nc.gpsimd.memset(q16[:, :, D:], 0.0)
        nc.gpsimd.memset(k16[:, :, D:], 0.0)
        self.q16, self.k16 = q16, k16
        self.qT = qkt_pool.tile([128, S], F16, tag="qT", name=f"qT{h}")
        self.kT = qkt_pool.tile([128, S], F16, tag="kT", name=f"kT{h}")
        nc.gpsimd.memset(self.qT[D:, :], 0.0)
        nc.gpsimd.memset(self.kT[D:, :], 0.0)
        hn = NQ // 2
        for i in range(2):
            ns = slice(i * hn, (i + 1) * hn)
            for nat, st16, dram_src in (
                (q_nat, q16, pools["q_dram"]),
                (k_nat, k16, pools["k_dram"]),
            ):
                nc.sync.dma_start(
                    nat[:, ns, :],
                    dram_src[h].rearrange("(n p) d -> p n d", p=128)[:, ns, :],
                )
                nc.vector.tensor_copy(st16[:, ns, :D], nat[:, ns, :])
            if i == 0:
                nc.sync.dma_start(
                    v_nat[:],
                    pools["v_dram"][h].rearrange("(n p) d -> p n d", p=128),
                )
            if i == 0:
                # kT tile 0 + qT tiles 0-7 unblock the first scores matmul
                order = [("k", 0)] + [("q", n) for n in range(hn)] \
                    + [("k", n) for n in range(1, hn)]
            else:
                order = [(kind, n) for kind in ("q", "k")
                         for n in range(i * hn, (i + 1) * hn)]
            for kind, n in order:
                self.emit_transpose(kind, n)
            if i == 0:
                v1 = head_pool.tile([128, NQ, D + 1], F16, tag="v1", name=f"v1_{h}")
                nc.vector.tensor_copy(
                    v1[:, :, D:].rearrange("p n one -> p (n one)"),
                    pools["ones"][:],
                )
                nc.vector.tensor_copy(v1[:, :, :D], v_nat[:])
                self.v1 = v1

    def start_dma(self, groups=None):
        """Emit the staging pipeline.  All DMA triggers run on the in-order
        SP queue, so a trigger whose dependency is unmet BLOCKS every DMA
        behind it; the emission order below is arranged so each trigger's
        producer has (almost) always completed by the time SP reaches it.

        groups: list of lists of (kind, n0, n1) chunk descriptors; each
        group emits load -> cast -> store -> transpose for its chunks in
        dependency-pipelined order.  "v" entries load V and build [V|1].
        """
        nc, pools, h = self.ctx["nc"], self.ctx, self.h
        head_pool = pools["head_pool"]
        dram_pool = pools["dram_pool"]
        qkt_pool = pools["qkt_pool"]

        q_nat = head_pool.tile([128, NQ, D], F32, tag="q_nat", name=f"q_nat{h}")
        k_nat = head_pool.tile([128, NQ, D], F32, tag="k_nat", name=f"k_nat{h}")
        v_nat = head_pool.tile([128, NQ, D], F32, tag="v_nat", name=f"v_nat{h}")
        # fp16 staging with d padded 64->128; pad columns stay zero across
        # slot reuse (each head only rewrites cols 0..63), cleared on the
        # first two uses (one per pool slot).
        q16 = head_pool.tile([128, NQ, 128], F16, tag="q16", name=f"q16_{h}")
        k16 = head_pool.tile([128, NQ, 128], F16, tag="k16", name=f"k16_{h}")
        if h < 2:
            # gpsimd (idle engine) so the DVE cast chain isn't delayed
            nc.gpsimd.memset(q16[:, :, D:], 0.0)
            nc.gpsimd.memset(k16[:, :, D:], 0.0)
        qdr = dram_pool.tile([S, 128], F16, tag="qdr", name=f"qdr{h}")
        kdr = dram_pool.tile([S, 128], F16, tag="kdr", name=f"kdr{h}")
        self.qT = qkt_pool.tile([128, S], F16, tag="qT", name=f"qT{h}")
        self.kT = qkt_pool.tile([128, S], F16, tag="kT", name=f"kT{h}")

        if groups is None:
            groups = [[("q", 0, NQ), ("k", 0, NQ), ("v", 0, NQ)]]

        def tensors(kind):
            return {
                "q": (q_nat, q16, qdr, self.qT, pools["q_dram"]),
                "k": (k_nat, k16, kdr, self.kT, pools["k_dram"]),
            }[kind]

        for group in groups:
            # stage 1: loads (wait-free on SP) + casts (DVE)
            for kind, n0, n1 in group:
                if kind == "v":
                    nc.sync.dma_start(
                        v_nat[:],
                        pools["v_dram"][h].rearrange("(n p) d -> p n d", p=128),
                    )
                    v1 = head_pool.tile([128, NQ, D + 1], F16, tag="v1", name=f"v1_{h}")
                    nc.vector.tensor_copy(
                        v1[:, :, D:].rearrange("p n one -> p (n one)"),
                        pools["ones"][:],
                    )
                    nc.vector.tensor_copy(v1[:, :, :D], v_nat[:])
                    self.v1 = v1
                    continue
                nat, st16, dr, tT, dram_src = tensors(kind)
                ns = slice(n0, n1)
                nc.sync.dma_start(
                    nat[:, ns, :],
                    dram_src[h].rearrange("(n p) d -> p n d", p=128)[:, ns, :],
                )
                nc.vector.tensor_copy(st16[:, ns, :D], nat[:, ns, :])
            # stage 2: bounce stores (wait on casts, usually met)
            for kind, n0, n1 in group:
                if kind == "v":
                    continue
                nat, st16, dr, tT, dram_src = tensors(kind)
                ns = slice(n0, n1)
                nc.sync.dma_start(
                    dr[:].rearrange("(n p) c -> p n c", p=128)[:, ns, :],
                    st16[:, ns, :],
                )
            # stage 3: X-bar transposes (wait on stores, usually met)
            for kind, n0, n1 in group:
                if kind == "v":
                    continue
                nat, st16, dr, tT, dram_src = tensors(kind)
                rs = slice(n0 * 128, n1 * 128)
                nc.sync.dma_start_transpose(tT[:, rs], dr[rs, :])


def _attention(tc):
    nc = tc.nc
    q_dram = nc.dram_tensor("query", [HPC, S, D], F32, kind="ExternalInput").ap()
    k_dram = nc.dram_tensor("key", [HPC, S, D], F32, kind="ExternalInput").ap()
    v_dram = nc.dram_tensor("value", [HPC, S, D], F32, kind="ExternalInput").ap()
    o_dram = nc.dram_tensor("out", [HPC, S, D], F32, kind="ExternalOutput").ap()

    with (
        tc.tile_pool(name="const", bufs=1) as const_pool,
        tc.tile_pool(name="head_io", bufs=2) as head_pool,
        tc.tile_pool(name="qkt", bufs=2) as qkt_pool,
        tc.tile_pool(name="et", bufs=4) as et_pool,
        tc.tile_pool(name="eti", bufs=2) as eti_pool,
        tc.tile_pool(name="epi", bufs=2) as epi_pool,
        tc.tile_pool(name="dram", bufs=2, space="DRAM") as dram_pool,
        tc.tile_pool(name="ps_s", bufs=2, space="PSUM") as ps_s_pool,
        tc.tile_pool(name="ps_f", bufs=1, space="PSUM") as ps_f_pool,
        tc.tile_pool(name="ps_o", bufs=1, space="PSUM") as ps_o_pool,
    ):
        ident16 = const_pool.tile([128, 128], F16)
        make_identity(nc, ident16[:])
        ones = const_pool.tile([128, NQ], F16)
        nc.vector.memset(ones[:], 1.0)

        ctx = {
            "nc": nc, "q_dram": q_dram, "k_dram": k_dram, "v_dram": v_dram,
            "head_pool": head_pool, "qkt_pool": qkt_pool,
            "ps_f_pool": ps_f_pool, "dram_pool": dram_pool,
            "ident16": ident16, "ones": ones,
        }

        heads = [_HeadInputs(ctx, h) for h in range(HPC)]
        heads[0].start_dma_split()

        def emit_scores(hd, qh, k, s_ps):
            for b in range(NB):
                q0 = qh * QH + b * 512
                nc.tensor.matmul(
                    s_ps[:, b * 512:(b + 1) * 512],
                    hd.kT[:, k * 128:(k + 1) * 128],
                    hd.qT[:, q0:q0 + 512],
                    start=True, stop=True,
                )

        def emit_pv(hd, oT, k, et_ap):
            for b in range(NB):
                nc.tensor.matmul(
                    oT[:, b * 512:(b + 1) * 512],
                    hd.v1[:, k, :],
                    et_ap[:, b * 512:(b + 1) * 512],
                    start=(k == 0), stop=(k == NQ - 1),
                )

        def epi_copy(st):
            """Stage A: evict oT PSUM -> fp16 SBUF (DVE)."""
            st["oT16"] = epi_pool.tile([80, QH], F16, tag="oT16", name="oT16")
            nc.vector.tensor_copy(st["oT16"][:D + 1, :], st["oT"][:])

        def epi_transpose(st):
            """Stage B: one X-bar transpose [80, QH] -> [128, 8, 80]."""
            st["trT"] = epi_pool.tile([128, QH // 128, 80], F16, tag="trT", name="trT")
            nc.sync.dma_start_transpose(st["trT"][:], st["oT16"][:])

        def epi_normalize(st):
            """Stage C: per-q reciprocal normalize (DVE) + output DMA."""
            h, qh, trT = st["h"], st["qh"], st["trT"]
            den = epi_pool.tile([128, QH // 128], F32, tag="den", name="den")
            nc.vector.tensor_scalar_add(den[:], trT[:, :, D], 1.0)
            rec = epi_pool.tile([128, QH // 128], F32, tag="rec", name="rec")
            nc.vector.reciprocal(rec[:], den[:])
            o_sb = epi_pool.tile([128, QH // 128, D], F32, tag="o_sb", name="o_sb")
            for j in range(QH // 128):
                nc.vector.tensor_scalar_mul(o_sb[:, j, :], trT[:, j, :D], rec[:, j:j + 1])
            nc.sync.dma_start(
                o_dram[h].rearrange("(n p) d -> p n d", p=128)[:, qh * 8:(qh + 1) * 8, :],
                o_sb[:],
            )

        pending_epi = []
        passes = [(h, qh) for h in range(HPC) for qh in range(S // QH)]
        s_carry = None
        for idx, (h, qh) in enumerate(passes):
            hd = heads[h]
            # prefetch the next head's staging a full pass ahead
            if qh == 0 and h + 1 < HPC:
                heads[h + 1].start_dma()

            oT = ps_o_pool.tile([D + 1, QH], F32, tag="oT", name="oT")
            s_tiles = {}
            if s_carry is not None:
                s_tiles[0] = s_carry
                s_carry = None
            else:
                s_tiles[0] = ps_s_pool.tile([128, QH], F32, tag="s", name="s0")
                emit_scores(hd, qh, 0, s_tiles[0])

            def alloc_s(tgt):
                if tgt in OFF:
                    return ps_f_pool.tile([128, QH], F32, tag="f", name=f"sf{tgt}")
                return ps_s_pool.tile([128, QH], F32, tag="s", name=f"s{tgt}")
            for k in range(NQ):
                if k in OFF:
                    eti = eti_pool.tile([128, QH], I16, tag="eti", name=f"eti{k}")
                    nc.vector.tensor_scalar(
                        eti[:], s_tiles[k][:], EXP_A, EXP_B,
                        mybir.AluOpType.mult, mybir.AluOpType.add,
                    )
                    et_ap = eti[:].bitcast(F16)
                else:
                    et = et_pool.tile([128, QH], F16, tag="et", name=f"et{k}")
                    nc.scalar.activation(et[:], s_tiles[k][:], EXP, scale=SCALE)
                    et_ap = et[:]
                if k + 1 < NQ:
                    s_tiles[k + 1] = alloc_s(k + 1)
                    emit_scores(hd, qh, k + 1, s_tiles[k + 1])
                elif idx + 1 < len(passes):
                    # hoist the next pass's first scores into this pass's
                    # tail so the ScalarEngine never idles at the boundary
                    nh, nqh = passes[idx + 1]
                    s_carry = ps_s_pool.tile([128, QH], F32, tag="s", name="sc")
                    emit_scores(heads[nh], nqh, 0, s_carry)
                # drain the previous pass's epilogue in stages (placed just
                # after OFF tiles so the DVE work never delays an eti) so
                # each DMA trigger's dependency is met when the in-order SP
                # queue reaches it
                if pending_epi:
                    if k == 3:
                        epi_copy(pending_epi[0])
                    elif k == 5:
                        epi_transpose(pending_epi[0])
                    elif k == 8:
                        epi_normalize(pending_epi.pop(0))
                emit_pv(hd, oT, k, et_ap)
                del s_tiles[k]
            pending_epi.append({"h": h, "qh": qh, "oT": oT})
        for st in pending_epi:
            epi_copy(st)
            epi_transpose(st)
            epi_normalize(st)


_NC_CACHE = None
_TRACE_READY = False


def _enable_tracing():
    """Register the NTFF profile hook that this image's antenv lacks, and
    keep profiling artifacts local instead of uploading to a bucket."""
    global _TRACE_READY
    if _TRACE_READY:
        return
    import sys
    import types

    import antenv
    import concourse.bass_utils as bu
    from trn_agent_boot.trn_boot import _ntff_profile_via_ctypes

    if "antenv.axon_hooks" not in sys.modules:
        mod = types.ModuleType("antenv.axon_hooks")
        mod._hook = None

        def set_axon_ntff_profile_hook(h):
            mod._hook = h

        def get_axon_ntff_profile_hook():
            return mod._hook

        mod.set_axon_ntff_profile_hook = set_axon_ntff_profile_hook
        mod.get_axon_ntff_profile_hook = get_axon_ntff_profile_hook
        sys.modules["antenv.axon_hooks"] = mod
        antenv.axon_hooks = mod

    hooks = sys.modules["antenv.axon_hooks"]
    if hooks.get_axon_ntff_profile_hook() is None:
        hooks.set_axon_ntff_profile_hook(
            _ntff_profile_via_ctypes("/opt/axon/libaxon_pjrt.so")
        )
    bu.upload_artifacts = lambda tmpdir: tmpdir
    _TRACE_READY = True


def _build():
    global _NC_CACHE
    if _NC_CACHE is None:
        nc = bacc.Bacc("TRN2", target_bir_lowering=False, debug=False)
        with tile.TileContext(nc) as tc:
            _attention(tc)
        nc.compile()
        _NC_CACHE = nc
    return _NC_CACHE


def _run(query, key, value, trace=False, tmpdir=None):
    if trace:
        _enable_tracing()
    q = np.ascontiguousarray(np.asarray(query, dtype=np.float32).reshape(B * H, S, D))
    k = np.ascontiguousarray(np.asarray(key, dtype=np.float32).reshape(B * H, S, D))
    v = np.ascontiguousarray(np.asarray(value, dtype=np.float32).reshape(B * H, S, D))
    in_maps = [
        {
            "query": q[c * HPC:(c + 1) * HPC],
            "key": k[c * HPC:(c + 1) * HPC],
            "value": v[c * HPC:(c + 1) * HPC],
        }
        for c in range(N_CORES)
    ]
    nc = _build()
    res = run_bass_kernel_spmd(
        nc, in_maps, core_ids=list(range(N_CORES)), trace=trace, tmpdir=tmpdir
    )
    out = np.stack([res.results[c]["out"] for c in range(N_CORES)])  # [8, HPC, S, D]
    return out.reshape(B, H, S, D), res


def kernel(query, key, value):
    out, _ = _run(query, key, value, trace=bool(int(os.environ.get("BASS_TRACE", "0"))))
    return out
